# revision 10
# baseline (speedup 1.0000x reference)
"""DCGRU cell Trainium2 kernel.

Math (per batch i):
  xs = [input, state]                                  [N, 66]
  aggr[j] = S[j] @ xs          (J=4 supports)          [N, 66]
  r = sigmoid(sum_j aggr[j] @ Wr[j] + br)              [N, 64]
  u = sigmoid(sum_j aggr[j] @ Wu[j] + bu)
  xc = [input, r*state]
  c = tanh(sum_j (S[j] @ xc) @ Wc[j] + bc)
  out = u*state + (1-u)*c

Sharding: data-parallel over batch, 8 batches per core on 8 cores.
supports/weights replicated. No collectives.

Device kernel structure (per core, Bc=8):
  - The input-feature part (2 of 66 contraction features, ~3% of FLOPs) is
    precomputed on host: ruin = sum_j (S[j]@input) @ Wru[j][:2] per batch in
    [o, k] layout, cin likewise for Wc (+bc) in [k, (i,o)] layout.  The
    device S-contraction then runs with state-only 512-column moving
    operands (ap-512 matmuls, one PSUM bank per accumulation).
  - Phase 1: per k-group g (256 rows): aggr_state[j] = ST[j].T-block @ xst,
    PSUM [128, 512]; drain fp16; batch-PAIRED PE transposes ([128,128]
    blocks -> two batches stacked on partitions); projection with
    row-duplicated W_state; DVE adds ruin; sigmoid -> ru[i] [128(r|u), 256].
    rsT = r*stateT on DVE; y[j] = rsT-block @ Wc_state (pre-projection,
    PE ap-256) packed [m, (i,j,o)]; u transposed to k-layout on PE.
    All tail work for group g-1 is emission-interleaved into group g's
    matmul chunks so the PE never idles.
  - Phase 2: c_pre[k, (i,o)] = sum_{j,m} ST[j]-block @ y[j] accumulated over
    j AND m in one PSUM [128, 512] per k-block (ap-512); DVE adds cin (which
    carries the host input part + bc); tanh; GRU combine on DVE against
    xst (= state in k-layout, already resident) and u_ko; fp16 out per
    k-block, fully overlapped with remaining matmuls.
"""

import sys

if '/opt/trn_rl_repo' not in sys.path:
    sys.path.insert(0, '/opt/trn_rl_repo')

import numpy as np

B, N, IN, OUT, J = 64, 2048, 2, 64, 4
NCORES = 8
BC = B // NCORES            # 8 batches per core
P = 128
NMB = N // P                # 16 m blocks
NG = 8                      # k groups
KBG = 2                     # k blocks (128) per group
GK = KBG * P                # 256 k per group
CB = BC * OUT               # 512 moving columns (state part)
MBQ = 8                     # m blocks per ST tile
ST_BUFS = 9

_CACHE = {}


def _build_module():
    import concourse.tile as tile
    import concourse.mybir as mybir
    from concourse import bacc
    from concourse.masks import make_identity

    f32 = mybir.dt.float32
    fp16 = mybir.dt.float16
    AF = mybir.ActivationFunctionType

    nc = bacc.Bacc("TRN2", target_bir_lowering=False, debug=False,
                   num_devices=1)

    st_d = nc.dram_tensor("st", [J, N, N], fp16, kind="ExternalInput").ap()
    xst_d = nc.dram_tensor("xst", [N, CB], fp16, kind="ExternalInput").ap()
    stt_d = nc.dram_tensor("stt", [BC, OUT, N], fp16, kind="ExternalInput").ap()
    ruin_d = nc.dram_tensor("ruin", [BC, 2 * OUT, N], fp16,
                            kind="ExternalInput").ap()
    cin_d = nc.dram_tensor("cin", [N, CB], fp16, kind="ExternalInput").ap()
    wst_d = nc.dram_tensor("wst", [P, 2 * P], fp16, kind="ExternalInput").ap()
    wca_d = nc.dram_tensor("wca", [OUT, J * OUT], fp16,
                           kind="ExternalInput").ap()
    bru_d = nc.dram_tensor("bru", [2 * OUT, 1], f32, kind="ExternalInput").ap()
    out_d = nc.dram_tensor("outk", [N, CB], fp16, kind="ExternalOutput").ap()

    from contextlib import ExitStack

    with tile.TileContext(nc) as tc, ExitStack() as es:
            pool = lambda name, bufs, **kw: es.enter_context(
                tc.tile_pool(name=name, bufs=bufs, **kw))
            const_pool = pool("const", 1)
            xs_pool = pool("xst", NMB)
            st_pool = pool("stp", ST_BUFS)
            aggsb_pool = pool("aggsb", 8)
            aggTg_pool = pool("aggTg", 2)
            ru_pool = pool("ru", 10)
            xcT_pool = pool("xcT", 10)
            stsl_pool = pool("stsl", 16)
            ruinsl_pool = pool("ruinsl", 16)
            y_pool = pool("ysb", NMB)
            uko_pool = pool("uko", 1)
            cin_pool = pool("cin", 4)
            csb_pool = pool("csb", 3)
            tmp_pool = pool("tmp", 6)
            agg_ps = pool("aggps", 3, space="PSUM")
            pp_ps = pool("ppps", 2, space="PSUM")
            y_ps = pool("yps", 2, space="PSUM")

            ident = const_pool.tile([P, P], fp16, tag="ident")
            make_identity(nc, ident[:])

            wst_t = const_pool.tile([P, 2 * P], fp16, tag="wst")
            wca_t = const_pool.tile([OUT, J * OUT], fp16, tag="wca")
            bru_t = const_pool.tile([2 * OUT, 1], f32, tag="bru")
            u_ko = uko_pool.tile([P, NMB * CB], fp16, tag="uko")

            # ---- DMA helpers ------------------------------------------
            st_tiles = {}          # (phase, g, j, mq) -> tile

            def st_dma(phase, g, j, mq):
                t = st_pool.tile([P, MBQ, GK], fp16, tag="st", name=f"st{phase}_{g}_{j}_{mq}")
                src = st_d[j, mq * MBQ * P:(mq + 1) * MBQ * P,
                           g * GK:(g + 1) * GK]
                src = src.rearrange("(q p) k -> p q k", p=P)
                nc.sync.dma_start(t[:], src)
                st_tiles[(phase, g, j, mq)] = t

            # phase-1 issue order (j-major); phase-2 g2=0 needs mq0 of all j
            # first (mb-major accumulation), later groups j-major again.
            P1_ORDER = [(j, mq) for j in range(J) for mq in range(2)]
            P2_FIRST_ORDER = [(j, 0) for j in range(J)] + \
                             [(j, 1) for j in range(J)]

            def paced_st(g, s):
                """Issue the s-th ST prefetch while group g is computing."""
                if g + 1 < NG:
                    j, mq = P1_ORDER[s]
                    st_dma(1, g + 1, j, mq)
                else:
                    j, mq = P2_FIRST_ORDER[s]
                    st_dma(2, 0, j, mq)

            def paced_st2(g2, s):
                if g2 + 1 < NG:
                    j, mq = P1_ORDER[s]
                    st_dma(2, g2 + 1, j, mq)

            stsl_t = {}
            ruin_t = {}

            def tail_prefetch(g):
                """stT / ruin slices needed by tail(g); issued on SP."""
                for i in range(BC):
                    t = stsl_pool.tile([OUT, GK], fp16, tag="stsl", name=f"stsl{i}_{g}")
                    nc.sync.dma_start(t[:], stt_d[i, :, g * GK:(g + 1) * GK])
                    stsl_t[(i, g)] = t
                    t2 = ruinsl_pool.tile([2 * OUT, GK], fp16, tag="ruinsl", name=f"ruin{i}_{g}")
                    nc.sync.dma_start(t2[:],
                                      ruin_d[i, :, g * GK:(g + 1) * GK])
                    ruin_t[(i, g)] = t2

            cin_t = {}

            def cin_prefetch(g2):
                for kb in range(KBG):
                    kbg = g2 * KBG + kb
                    t = cin_pool.tile([P, CB], fp16, tag="cin", name=f"cin{kbg}")
                    nc.scalar.dma_start(t[:], cin_d[kbg * P:(kbg + 1) * P, :])
                    cin_t[kbg] = t

            # ---- initial DMAs -----------------------------------------
            xs_tiles = [None] * NMB

            def xs_dma(mb):
                t = xs_pool.tile([P, CB], fp16, tag="xst", name=f"xst{mb}")
                nc.sync.dma_start(t[:], xst_d[mb * P:(mb + 1) * P, :])
                xs_tiles[mb] = t

            st_dma(1, 0, 0, 0)
            for mb in range(MBQ):
                xs_dma(mb)
            st_dma(1, 0, 0, 1)
            for mb in range(MBQ, NMB):
                xs_dma(mb)
            nc.scalar.dma_start(wst_t[:], wst_d[:])
            nc.scalar.dma_start(wca_t[:], wca_d[:])
            nc.scalar.dma_start(bru_t[:], bru_d[:])
            for j, mq in P1_ORDER[2:]:
                st_dma(1, 0, j, mq)
            tail_prefetch(0)
            cin_prefetch(0)

            agg_sb = {}
            aggT_g = {}            # g -> [128, 4096] fp16 (i, jp, kb, k)            # (g, j, kb) -> [128, 512] fp16
            y_sb = {}              # mb -> [128, 2048] fp16 (cols i,j,o)

            # ---- phase-1 chunk: 16 ap-512 matmuls + drain -------------
            def p1_chunk(g, s):
                j, kb = s // 2, s % 2
                pst = agg_ps.tile([P, CB], f32, tag="aggps", name=f"agg{g}_{s}")
                for mb in range(NMB):
                    mq, ml = divmod(mb, MBQ)
                    lhsT = st_tiles[(1, g, j, mq)][:, ml,
                                                   kb * P:(kb + 1) * P]
                    nc.tensor.matmul(pst[:], lhsT, xs_tiles[mb][:],
                                     start=(mb == 0), stop=(mb == NMB - 1))
                jp, jh = divmod(j, 2)
                if jh == 0 and (g, jp, kb) not in agg_sb:
                    agg_sb[(g, jp, kb)] = aggsb_pool.tile(
                        [P, 2 * CB], fp16, tag="aggsb", name=f"asb{g}_{jp}_{kb}")
                asb = agg_sb[(g, jp, kb)]
                dst = asb[:].rearrange("p (i jh o) -> p i jh o",
                                       i=BC, jh=2)[:, :, jh, :]
                srcv = pst[:].rearrange("p (i o) -> p i o", i=BC)
                if s % 2 == 0:
                    nc.vector.tensor_copy(dst, srcv)
                else:
                    nc.scalar.copy(dst, srcv)
                if jh == 1:
                    # both j-halves drained: XBAR-transpose straight to SBUF
                    if g not in aggT_g:
                        aggT_g[g] = aggTg_pool.tile(
                            [P, BC * 2 * GK], fp16, tag="aggTg",
                            name=f"aggTg{g}")
                    # out block for (jp, kb): [128, 8(i), 128] at offset
                    out3 = aggT_g[g][:].rearrange(
                        "p (i x) -> p i x", i=BC)[
                        :, :, jp * GK + kb * P:jp * GK + (kb + 1) * P]
                    nc.scalar.dma_start_transpose(out3, asb[:])

            # ---- tail(g): transposes/proj/act/rsT/y/u for group g -----
            # emitted as a generator with 8 slices, interleaved into the
            # NEXT group's matmul chunks.
            def proj_act(g, i, pp_slice):
                for jp in range(2):
                    nc.tensor.matmul(
                        pp_slice,
                        wst_t[:, jp * P:(jp + 1) * P],
                        aggT_sl[i][jp],
                        start=(jp == 0), stop=(jp == 1),
                        skip_group_check=True)
                nc.vector.tensor_add(pp_slice, pp_slice,
                                     ruin_t[(i, g)][:])
                ru = ru_pool.tile([P, GK], fp16, tag="ru", name=f"ru{g}_{i}")
                nc.scalar.activation(ru[:], pp_slice, AF.Sigmoid,
                                     bias=bru_t[:, 0:1])
                xct = xcT_pool.tile([OUT, GK], fp16, tag="xcT", name=f"xcT{g}_{i}")
                nc.vector.tensor_mul(xct[:], ru[0:OUT, :],
                                     stsl_t[(i, g)][:])
                u3 = u_ko[:].rearrange("p (kbg c) -> p kbg c", c=CB)[
                    :, g * KBG:(g + 1) * KBG, i * OUT:(i + 1) * OUT]
                nc.scalar.dma_start_transpose(u3, ru[OUT:2 * OUT, :])
                ru_t[i] = ru
                xcT_t[i] = xct

            def y_mms(g, i, mbl):
                mb = g * KBG + mbl
                sl = y_slot(i)
                nc.tensor.matmul(
                    sl, xcT_t[i][:, mbl * P:(mbl + 1) * P], wca_t[:],
                    start=True, stop=True, skip_group_check=True)

            ru_t = {}
            xcT_t = {}
            aggT_sl = {}
            y_cur = {}

            def y_slot(i):
                # two batches share one [128, 512] f32 psum tile
                if i % 2 == 0:
                    y_cur['n'] = y_cur.get('n', 0) + 1
                    y_cur['t'] = y_ps.tile([P, 2 * J * OUT], f32, tag="yps", name=f"yps{y_cur['n']}")
                t = y_cur['t']
                return t[:, (i % 2) * J * OUT:(i % 2 + 1) * J * OUT]

            def tail(g):
                for i in range(BC):
                    aggT_sl[i] = [
                        aggT_g[g][:, i * 2 * GK + jp * GK:
                                  i * 2 * GK + (jp + 1) * GK]
                        for jp in range(2)]

                pp_tiles = {}

                def pp_slice(i):
                    if i % 2 == 0:
                        pp_tiles[i // 2] = pp_ps.tile([P, 2 * GK], f32,
                                                      tag="ppps", name=f"pp{g}_{i // 2}")
                    t = pp_tiles[i // 2]
                    return t[:, (i % 2) * GK:(i % 2 + 1) * GK]

                yield
                proj_act(g, 0, pp_slice(0))
                yield
                proj_act(g, 1, pp_slice(1))
                yield
                proj_act(g, 2, pp_slice(2))
                yield
                proj_act(g, 3, pp_slice(3))
                proj_act(g, 4, pp_slice(4))
                yield
                proj_act(g, 5, pp_slice(5))
                proj_act(g, 6, pp_slice(6))
                yield
                proj_act(g, 7, pp_slice(7))
                yield
                # y pre-projection (both m-blocks of this group)
                for mbl in range(KBG):
                    yt = y_pool.tile([P, BC * J * OUT], fp16,
                                     tag="ysb", name=f"y{g * KBG + mbl}")
                    y_sb[g * KBG + mbl] = yt
                    for i in range(BC):
                        y_mms(g, i, mbl)
                        if i % 2 == 1:
                            t = y_cur['t']
                            sl = yt[:, (i - 1) * J * OUT:(i + 1) * J * OUT]
                            if (i // 2 + mbl) % 2 == 0:
                                nc.vector.tensor_copy(sl, t[:])
                            else:
                                nc.scalar.copy(sl, t[:])
                yield

            # ---- phase-2 combine for one k-block ----------------------
            def combine(kbg, cps):
                nc.vector.tensor_add(cps, cps, cin_t[kbg][:])
                c = csb_pool.tile([P, CB], fp16, tag="csb", name=f"c{kbg}")
                nc.scalar.activation(c[:], cps, AF.Tanh)
                t1 = tmp_pool.tile([P, CB], fp16, tag="tmp", name=f"t1_{kbg}")
                nc.vector.tensor_sub(t1[:], xs_tiles[kbg][:], c[:])
                t2 = tmp_pool.tile([P, CB], fp16, tag="tmp", name=f"t2_{kbg}")
                nc.vector.tensor_mul(
                    t2[:], u_ko[:, kbg * CB:(kbg + 1) * CB], t1[:])
                t3 = tmp_pool.tile([P, CB], fp16, tag="tmp", name=f"t3_{kbg}")
                nc.vector.tensor_add(t3[:], c[:], t2[:])
                nc.scalar.dma_start(out_d[kbg * P:(kbg + 1) * P, :], t3[:])

            # ================= phase 1 =================
            tail_gen = None
            for g in range(NG):
                for s in range(2 * J):
                    paced_st(g, s)
                    p1_chunk(g, s)
                    if tail_gen is not None:
                        next(tail_gen, None)
                if g + 1 < NG:
                    tail_prefetch(g + 1)
                tail_gen = tail(g)

            # ================= phase 2 =================
            for g2 in range(NG):
                cps = [agg_ps.tile([P, CB], f32, tag="aggps",
                                   name=f"c{g2}_{kb}")
                       for kb in range(KBG)]
                if g2 + 1 < NG:
                    cin_prefetch(g2 + 1)
                if g2 == 0:
                    # mb-major accumulation; interleave tail(7)
                    for s in range(8):
                        next(tail_gen, None)
                        paced_st2(0, s)
                        for kb in range(KBG):
                            for j in range(J):
                                for mb in (2 * s, 2 * s + 1):
                                    mq, ml = divmod(mb, MBQ)
                                    rhs = y_sb[mb][:].rearrange(
                                        "p (i c) -> p i c", i=BC)[
                                        :, :, j * OUT:(j + 1) * OUT]
                                    nc.tensor.matmul(
                                        cps[kb][:],
                                        st_tiles[(2, 0, j, mq)][
                                            :, ml, kb * P:(kb + 1) * P],
                                        rhs,
                                        start=(s == 0 and j == 0
                                               and mb == 0),
                                        stop=(s == 7 and j == J - 1
                                              and mb == NMB - 1))
                    for kb in range(KBG):
                        combine(g2 * KBG + kb, cps[kb][:])
                else:
                    for s in range(8):
                        paced_st2(g2, s)
                        kb, j = s // 4, s % 4
                        for mb in range(NMB):
                            mq, ml = divmod(mb, MBQ)
                            rhs = y_sb[mb][:].rearrange(
                                "p (i c) -> p i c", i=BC)[
                                :, :, j * OUT:(j + 1) * OUT]
                            nc.tensor.matmul(
                                cps[kb][:],
                                st_tiles[(2, g2, j, mq)][
                                    :, ml, kb * P:(kb + 1) * P],
                                rhs,
                                start=(j == 0 and mb == 0),
                                stop=(j == J - 1 and mb == NMB - 1))
                        if s == 3:
                            combine(g2 * KBG, cps[0][:])
                        elif s == 7:
                            combine(g2 * KBG + 1, cps[1][:])

    nc.compile()
    return nc


def _get_module():
    if "nc" not in _CACHE:
        _CACHE["nc"] = _build_module()
    return _CACHE["nc"]


def kernel(input, state, supports, Wr, br, Wu, bu, Wc, bc):
    input = np.asarray(input, np.float32)
    state = np.asarray(state, np.float32)
    supports = np.asarray(supports, np.float32)
    Wr = np.asarray(Wr, np.float32)
    br = np.asarray(br, np.float32)
    Wu = np.asarray(Wu, np.float32)
    bu = np.asarray(bu, np.float32)
    Wc = np.asarray(Wc, np.float32)
    bc = np.asarray(bc, np.float32)

    from concourse.bass_utils import run_bass_kernel_spmd

    nc = _get_module()

    st_host = np.ascontiguousarray(
        supports.transpose(0, 2, 1).astype(np.float16))

    Wru = np.concatenate([Wr, Wu], axis=2)          # [J, 66, 128]
    W_state = Wru[:, IN:, :]                        # [J, 64, 128]
    W_in = Wru[:, :IN, :]                           # [J, 2, 128]
    Wc_state = Wc[:, IN:, :]                        # [J, 64, 64]
    Wc_in = Wc[:, :IN, :]                           # [J, 2, 64]

    wst = np.empty((P, 2 * P), np.float16)
    wca = np.empty((OUT, J * OUT), np.float16)
    for jp in range(2):
        wst[:OUT, jp * P:(jp + 1) * P] = W_state[2 * jp]
        wst[OUT:, jp * P:(jp + 1) * P] = W_state[2 * jp + 1]
    for j in range(J):
        wca[:, j * OUT:(j + 1) * OUT] = Wc_state[j]
    bru = np.concatenate([br, bu]).reshape(2 * OUT, 1).astype(np.float32)

    # host-side input-feature part: AG[j] = S[j] @ input  (3% of FLOPs)
    X = np.ascontiguousarray(
        input.transpose(1, 0, 2).reshape(N, B * IN))
    AG = np.stack([supports[j] @ X for j in range(J)])  # [J, N, B*IN]
    AG4 = AG.reshape(J, N, B, IN)
    ruin_full = np.einsum('jkbf,jfo->bok', AG4, W_in)   # [B, 128, N]
    cin_full = np.einsum('jkbf,jfo->bko', AG4, Wc_in) + bc  # [B, N, 64]

    in_maps = []
    for c in range(NCORES):
        sl = slice(c * BC, (c + 1) * BC)
        st_c = state[sl]                              # [BC, N, OUT]
        xst = np.ascontiguousarray(
            st_c.transpose(1, 0, 2).reshape(N, CB).astype(np.float16))
        stt = np.ascontiguousarray(
            st_c.transpose(0, 2, 1).astype(np.float16))
        ruin = np.ascontiguousarray(ruin_full[sl].astype(np.float16))
        cin = np.ascontiguousarray(
            cin_full[sl].transpose(1, 0, 2).reshape(N, CB)
            .astype(np.float16))
        in_maps.append({
            "st": st_host,
            "xst": xst,
            "stt": stt,
            "ruin": ruin,
            "cin": cin,
            "wst": wst,
            "wca": wca,
            "bru": bru,
        })

    import time
    t0 = time.monotonic()
    res = run_bass_kernel_spmd(nc, in_maps, core_ids=list(range(NCORES)))
    _CACHE["last_wall_s"] = time.monotonic() - t0

    out = np.empty((B, N, OUT), np.float32)
    for c in range(NCORES):
        outk = res.results[c]["outk"]                 # [N, 512] fp16
        for i in range(BC):
            out[c * BC + i] = outk[:, i * OUT:(i + 1) * OUT].astype(
                np.float32)
    return out


# revision 11
# speedup vs baseline: 1.3312x; 1.3312x over previous
"""DCGRU cell Trainium2 kernel.

Math (per batch i):
  xs = [input, state]                                  [N, 66]
  aggr[j] = S[j] @ xs          (J=4 supports)          [N, 66]
  r = sigmoid(sum_j aggr[j] @ Wr[j] + br)              [N, 64]
  u = sigmoid(sum_j aggr[j] @ Wu[j] + bu)
  xc = [input, r*state]
  c = tanh(sum_j (S[j] @ xc) @ Wc[j] + bc)
  out = u*state + (1-u)*c

Sharding: data-parallel over batch, 8 batches per core on 8 cores.
supports/weights replicated. No collectives.

Device kernel structure (per core, Bc=8):
  - The input-feature part (2 of 66 contraction features, ~3% of FLOPs) is
    precomputed on host: ruin = sum_j (S[j]@input) @ Wru[j][:2] per batch in
    [o, k] layout, cin likewise for Wc (+bc) in [k, (i,o)] layout.  The
    device S-contraction then runs with state-only 512-column moving
    operands (ap-512 matmuls, one PSUM bank per accumulation).
  - Phase 1: per k-group g (256 rows): aggr_state[j] = ST[j].T-block @ xst,
    PSUM [128, 512]; drain fp16; batch-PAIRED PE transposes ([128,128]
    blocks -> two batches stacked on partitions); projection with
    row-duplicated W_state; DVE adds ruin; sigmoid -> ru[i] [128(r|u), 256].
    rsT = r*stateT on DVE; y[j] = rsT-block @ Wc_state (pre-projection,
    PE ap-256) packed [m, (i,j,o)]; u transposed to k-layout on PE.
    All tail work for group g-1 is emission-interleaved into group g's
    matmul chunks so the PE never idles.
  - Phase 2: c_pre[k, (i,o)] = sum_{j,m} ST[j]-block @ y[j] accumulated over
    j AND m in one PSUM [128, 512] per k-block (ap-512); DVE adds cin (which
    carries the host input part + bc); tanh; GRU combine on DVE against
    xst (= state in k-layout, already resident) and u_ko; fp16 out per
    k-block, fully overlapped with remaining matmuls.
"""

import sys

if '/opt/trn_rl_repo' not in sys.path:
    sys.path.insert(0, '/opt/trn_rl_repo')

import numpy as np

B, N, IN, OUT, J = 64, 2048, 2, 64, 4
NCORES = 8
BC = B // NCORES            # 8 batches per core
P = 128
NMB = N // P                # 16 m blocks
NG = 8                      # k groups
KBG = 2                     # k blocks (128) per group
GK = KBG * P                # 256 k per group
CB = BC * OUT               # 512 moving columns (state part)
MBQ = 8                     # m blocks per ST tile
ST_BUFS = 9

_CACHE = {}


def _build_module():
    import concourse.tile as tile
    import concourse.mybir as mybir
    from concourse import bacc
    from concourse.masks import make_identity

    f32 = mybir.dt.float32
    fp16 = mybir.dt.float16
    AF = mybir.ActivationFunctionType

    nc = bacc.Bacc("TRN2", target_bir_lowering=False, debug=False,
                   num_devices=1)

    st_d = nc.dram_tensor("st", [J, N, N], fp16, kind="ExternalInput").ap()
    xst_d = nc.dram_tensor("xst", [N, CB], fp16, kind="ExternalInput").ap()
    stt_d = nc.dram_tensor("stt", [BC, OUT, N], fp16, kind="ExternalInput").ap()
    ruin_d = nc.dram_tensor("ruin", [BC, 2 * OUT, N], fp16,
                            kind="ExternalInput").ap()
    cin_d = nc.dram_tensor("cin", [N, CB], fp16, kind="ExternalInput").ap()
    wst_d = nc.dram_tensor("wst", [P, 2 * P], fp16, kind="ExternalInput").ap()
    wca_d = nc.dram_tensor("wca", [OUT, J * OUT], fp16,
                           kind="ExternalInput").ap()
    bru_d = nc.dram_tensor("bru", [2 * OUT, 1], f32, kind="ExternalInput").ap()
    out_d = nc.dram_tensor("outk", [N, CB], fp16, kind="ExternalOutput").ap()

    from contextlib import ExitStack

    with tile.TileContext(nc) as tc, ExitStack() as es:
            pool = lambda name, bufs, **kw: es.enter_context(
                tc.tile_pool(name=name, bufs=bufs, **kw))
            const_pool = pool("const", 1)
            xs_pool = pool("xst", NMB)
            st_pool = pool("stp", ST_BUFS)
            aggsb_pool = pool("aggsb", 8)
            aggTg_pool = pool("aggTg", 2)
            ru_pool = pool("ru", 10)
            xcT_pool = pool("xcT", 10)
            stsl_pool = pool("stsl", 16)
            ruinsl_pool = pool("ruinsl", 16)
            y_pool = pool("ysb", NMB)
            uko_pool = pool("uko", 1)
            cin_pool = pool("cin", 4)
            csb_pool = pool("csb", 3)
            tmp_pool = pool("tmp", 6)
            agg_ps = pool("aggps", 3, space="PSUM")
            utp_ps = pool("utpps", 1, space="PSUM")
            pp_ps = pool("ppps", 2, space="PSUM")
            y_ps = pool("yps", 2, space="PSUM")

            ident = const_pool.tile([P, P], fp16, tag="ident")
            make_identity(nc, ident[:])

            wst_t = const_pool.tile([P, 2 * P], fp16, tag="wst")
            wca_t = const_pool.tile([OUT, J * OUT], fp16, tag="wca")
            bru_t = const_pool.tile([2 * OUT, 1], f32, tag="bru")
            u_ko = uko_pool.tile([P, NMB * CB], fp16, tag="uko")

            # ---- DMA helpers ------------------------------------------
            st_tiles = {}          # (phase, g, j, mq) -> tile

            def st_dma(phase, g, j, mq):
                t = st_pool.tile([P, MBQ, GK], fp16, tag="st", name=f"st{phase}_{g}_{j}_{mq}")
                src = st_d[j, mq * MBQ * P:(mq + 1) * MBQ * P,
                           g * GK:(g + 1) * GK]
                src = src.rearrange("(q p) k -> p q k", p=P)
                nc.sync.dma_start(t[:], src)
                st_tiles[(phase, g, j, mq)] = t

            # phase-1 issue order (j-major); phase-2 g2=0 needs mq0 of all j
            # first (mb-major accumulation), later groups j-major again.
            P1_ORDER = [(j, mq) for j in range(J) for mq in range(2)]
            P2_FIRST_ORDER = [(j, 0) for j in range(J)] + \
                             [(j, 1) for j in range(J)]

            def paced_st(g, s):
                """Issue the s-th ST prefetch while group g is computing."""
                if g + 1 < NG:
                    j, mq = P1_ORDER[s]
                    st_dma(1, g + 1, j, mq)
                else:
                    j, mq = P2_FIRST_ORDER[s]
                    st_dma(2, 0, j, mq)

            def paced_st2(g2, s):
                if g2 + 1 < NG:
                    j, mq = P1_ORDER[s]
                    st_dma(2, g2 + 1, j, mq)

            stsl_t = {}
            ruin_t = {}

            def tail_prefetch(g):
                """stT / ruin slices needed by tail(g); issued on SP."""
                for i in range(BC):
                    t = stsl_pool.tile([OUT, GK], fp16, tag="stsl", name=f"stsl{i}_{g}")
                    nc.sync.dma_start(t[:], stt_d[i, :, g * GK:(g + 1) * GK])
                    stsl_t[(i, g)] = t
                    t2 = ruinsl_pool.tile([2 * OUT, GK], fp16, tag="ruinsl", name=f"ruin{i}_{g}")
                    nc.sync.dma_start(t2[:],
                                      ruin_d[i, :, g * GK:(g + 1) * GK])
                    ruin_t[(i, g)] = t2

            cin_t = {}

            def cin_prefetch(g2):
                for kb in range(KBG):
                    kbg = g2 * KBG + kb
                    t = cin_pool.tile([P, CB], fp16, tag="cin", name=f"cin{kbg}")
                    nc.scalar.dma_start(t[:], cin_d[kbg * P:(kbg + 1) * P, :])
                    cin_t[kbg] = t

            # ---- initial DMAs -----------------------------------------
            xs_tiles = [None] * NMB

            def xs_dma(mb):
                t = xs_pool.tile([P, CB], fp16, tag="xst", name=f"xst{mb}")
                nc.sync.dma_start(t[:], xst_d[mb * P:(mb + 1) * P, :])
                xs_tiles[mb] = t

            st_dma(1, 0, 0, 0)
            for mb in range(MBQ):
                xs_dma(mb)
            st_dma(1, 0, 0, 1)
            for mb in range(MBQ, NMB):
                xs_dma(mb)
            nc.scalar.dma_start(wst_t[:], wst_d[:])
            nc.scalar.dma_start(wca_t[:], wca_d[:])
            nc.scalar.dma_start(bru_t[:], bru_d[:])
            for j, mq in P1_ORDER[2:]:
                st_dma(1, 0, j, mq)
            tail_prefetch(0)
            cin_prefetch(0)

            agg_sb = {}
            aggT_g = {}            # g -> [128, 4096] fp16 (i, jp, kb, k)            # (g, j, kb) -> [128, 512] fp16
            y_sb = {}              # mb -> [128, 2048] fp16 (cols i,j,o)

            # ---- phase-1 chunk: 16 ap-512 matmuls + drain -------------
            def p1_chunk(g, s):
                j, kb = s // 2, s % 2
                pst = agg_ps.tile([P, CB], f32, tag="aggps", name=f"agg{g}_{s}")
                for mb in range(NMB):
                    mq, ml = divmod(mb, MBQ)
                    lhsT = st_tiles[(1, g, j, mq)][:, ml,
                                                   kb * P:(kb + 1) * P]
                    nc.tensor.matmul(pst[:], lhsT, xs_tiles[mb][:],
                                     start=(mb == 0), stop=(mb == NMB - 1))
                jp, jh = divmod(j, 2)
                if jh == 0 and (g, jp, kb) not in agg_sb:
                    agg_sb[(g, jp, kb)] = aggsb_pool.tile(
                        [P, 2 * CB], fp16, tag="aggsb", name=f"asb{g}_{jp}_{kb}")
                asb = agg_sb[(g, jp, kb)]
                dst = asb[:].rearrange("p (i jh o) -> p i jh o",
                                       i=BC, jh=2)[:, :, jh, :]
                srcv = pst[:].rearrange("p (i o) -> p i o", i=BC)
                if s % 2 == 0:
                    nc.vector.tensor_copy(dst, srcv)
                else:
                    nc.scalar.copy(dst, srcv)


            # ---- tail(g): transposes/proj/act/rsT/y/u for group g -----
            # emitted as a generator with 8 slices, interleaved into the
            # NEXT group's matmul chunks.
            def proj_act(g, i, pp_slice):
                for jp in range(2):
                    nc.tensor.matmul(
                        pp_slice,
                        wst_t[:, jp * P:(jp + 1) * P],
                        aggT_sl[i][jp],
                        start=(jp == 0), stop=(jp == 1),
                        skip_group_check=True)
                nc.vector.tensor_add(pp_slice, pp_slice,
                                     ruin_t[(i, g)][:])
                ru = ru_pool.tile([P, GK], fp16, tag="ru", name=f"ru{g}_{i}")
                nc.scalar.activation(ru[:], pp_slice, AF.Sigmoid,
                                     bias=bru_t[:, 0:1])
                xct = xcT_pool.tile([OUT, GK], fp16, tag="xcT", name=f"xcT{g}_{i}")
                nc.vector.tensor_mul(xct[:], ru[0:OUT, :],
                                     stsl_t[(i, g)][:])
                ru_t[i] = ru
                xcT_t[i] = xct

            def y_mms(g, i, mbl):
                mb = g * KBG + mbl
                sl = y_slot(i)
                nc.tensor.matmul(
                    sl, xcT_t[i][:, mbl * P:(mbl + 1) * P], wca_t[:],
                    start=True, stop=True, skip_group_check=True)

            ru_t = {}
            xcT_t = {}
            aggT_sl = {}
            y_cur = {}

            def y_slot(i):
                # two batches share one [128, 512] f32 psum tile
                if i % 2 == 0:
                    y_cur['n'] = y_cur.get('n', 0) + 1
                    y_cur['t'] = y_ps.tile([P, 2 * J * OUT], f32, tag="yps", name=f"yps{y_cur['n']}")
                t = y_cur['t']
                return t[:, (i % 2) * J * OUT:(i % 2 + 1) * J * OUT]

            def tail(g):
                aggT_g[g] = aggTg_pool.tile(
                    [P, BC * 2 * GK], fp16, tag="aggTg", name=f"aggTg{g}")

                def agg_dmatp(jp, kb):
                    out3 = aggT_g[g][:].rearrange(
                        "p (i x) -> p i x", i=BC)[
                        :, :, jp * GK + kb * P:jp * GK + (kb + 1) * P]
                    nc.scalar.dma_start_transpose(
                        out3, agg_sb[(g, jp, kb)][:])

                for i in range(BC):
                    aggT_sl[i] = [
                        aggT_g[g][:, i * 2 * GK + jp * GK:
                                  i * 2 * GK + (jp + 1) * GK]
                        for jp in range(2)]

                pp_tiles = {}

                def pp_slice(i):
                    if i % 2 == 0:
                        pp_tiles[i // 2] = pp_ps.tile([P, 2 * GK], f32,
                                                      tag="ppps", name=f"pp{g}_{i // 2}")
                    t = pp_tiles[i // 2]
                    return t[:, (i % 2) * GK:(i % 2 + 1) * GK]

                agg_dmatp(0, 0)
                agg_dmatp(0, 1)
                yield
                agg_dmatp(1, 0)
                agg_dmatp(1, 1)
                yield
                proj_act(g, 0, pp_slice(0))
                proj_act(g, 1, pp_slice(1))
                yield
                proj_act(g, 2, pp_slice(2))
                proj_act(g, 3, pp_slice(3))
                yield
                proj_act(g, 4, pp_slice(4))
                proj_act(g, 5, pp_slice(5))
                yield
                proj_act(g, 6, pp_slice(6))
                proj_act(g, 7, pp_slice(7))
                yield
                # y pre-projection (both m-blocks of this group)
                for mbl in range(KBG):
                    yt = y_pool.tile([P, BC * J * OUT], fp16,
                                     tag="ysb", name=f"y{g * KBG + mbl}")
                    y_sb[g * KBG + mbl] = yt
                    for i in range(BC):
                        y_mms(g, i, mbl)
                        if i % 2 == 1:
                            t = y_cur['t']
                            sl = yt[:, (i - 1) * J * OUT:(i + 1) * J * OUT]
                            if (i // 2 + mbl) % 2 == 0:
                                nc.vector.tensor_copy(sl, t[:])
                            else:
                                nc.scalar.copy(sl, t[:])
                # u -> k-layout: PE transposes of ru rows 64:128
                utp = utp_ps.tile([P, KBG * CB], fp16, tag="utp",
                                  name=f"utp{g}")
                for kb in range(KBG):
                    for i in range(BC):
                        nc.tensor.transpose(
                            utp[:, kb * CB + i * OUT:kb * CB + (i + 1) * OUT],
                            ru_t[i][OUT:2 * OUT, kb * P:(kb + 1) * P],
                            ident[OUT:2 * OUT, OUT:2 * OUT])
                dst = u_ko[:, g * KBG * CB:(g + 1) * KBG * CB]
                nc.vector.tensor_copy(dst[:, 0:CB], utp[:, 0:CB])
                nc.scalar.copy(dst[:, CB:], utp[:, CB:])
                yield

            # ---- phase-2 combine for one k-block ----------------------
            def combine(kbg, cps):
                nc.vector.tensor_add(cps, cps, cin_t[kbg][:])
                c = csb_pool.tile([P, CB], fp16, tag="csb", name=f"c{kbg}")
                nc.scalar.activation(c[:], cps, AF.Tanh)
                t1 = tmp_pool.tile([P, CB], fp16, tag="tmp", name=f"t1_{kbg}")
                nc.vector.tensor_sub(t1[:], xs_tiles[kbg][:], c[:])
                t2 = tmp_pool.tile([P, CB], fp16, tag="tmp", name=f"t2_{kbg}")
                nc.vector.tensor_mul(
                    t2[:], u_ko[:, kbg * CB:(kbg + 1) * CB], t1[:])
                t3 = tmp_pool.tile([P, CB], fp16, tag="tmp", name=f"t3_{kbg}")
                nc.vector.tensor_add(t3[:], c[:], t2[:])
                nc.scalar.dma_start(out_d[kbg * P:(kbg + 1) * P, :], t3[:])

            # ================= phase 1 =================
            tail_gen = None
            for g in range(NG):
                for s in range(2 * J):
                    paced_st(g, s)
                    p1_chunk(g, s)
                    if tail_gen is not None:
                        next(tail_gen, None)
                if g + 1 < NG:
                    tail_prefetch(g + 1)
                tail_gen = tail(g)

            # ================= phase 2 =================
            for g2 in range(NG):
                cps = [agg_ps.tile([P, CB], f32, tag="aggps",
                                   name=f"c{g2}_{kb}")
                       for kb in range(KBG)]
                if g2 + 1 < NG:
                    cin_prefetch(g2 + 1)
                if g2 == 0:
                    # mb-major accumulation; interleave tail(7)
                    for s in range(8):
                        next(tail_gen, None)
                        paced_st2(0, s)
                        for kb in range(KBG):
                            for j in range(J):
                                for mb in (2 * s, 2 * s + 1):
                                    mq, ml = divmod(mb, MBQ)
                                    rhs = y_sb[mb][:].rearrange(
                                        "p (i c) -> p i c", i=BC)[
                                        :, :, j * OUT:(j + 1) * OUT]
                                    nc.tensor.matmul(
                                        cps[kb][:],
                                        st_tiles[(2, 0, j, mq)][
                                            :, ml, kb * P:(kb + 1) * P],
                                        rhs,
                                        start=(s == 0 and j == 0
                                               and mb == 0),
                                        stop=(s == 7 and j == J - 1
                                              and mb == NMB - 1))
                    for kb in range(KBG):
                        combine(g2 * KBG + kb, cps[kb][:])
                else:
                    for s in range(8):
                        paced_st2(g2, s)
                        kb, j = s // 4, s % 4
                        for mb in range(NMB):
                            mq, ml = divmod(mb, MBQ)
                            rhs = y_sb[mb][:].rearrange(
                                "p (i c) -> p i c", i=BC)[
                                :, :, j * OUT:(j + 1) * OUT]
                            nc.tensor.matmul(
                                cps[kb][:],
                                st_tiles[(2, g2, j, mq)][
                                    :, ml, kb * P:(kb + 1) * P],
                                rhs,
                                start=(j == 0 and mb == 0),
                                stop=(j == J - 1 and mb == NMB - 1))
                        if s == 3:
                            combine(g2 * KBG, cps[0][:])
                        elif s == 7:
                            combine(g2 * KBG + 1, cps[1][:])

    nc.compile()
    return nc


def _get_module():
    if "nc" not in _CACHE:
        _CACHE["nc"] = _build_module()
    return _CACHE["nc"]


def kernel(input, state, supports, Wr, br, Wu, bu, Wc, bc):
    input = np.asarray(input, np.float32)
    state = np.asarray(state, np.float32)
    supports = np.asarray(supports, np.float32)
    Wr = np.asarray(Wr, np.float32)
    br = np.asarray(br, np.float32)
    Wu = np.asarray(Wu, np.float32)
    bu = np.asarray(bu, np.float32)
    Wc = np.asarray(Wc, np.float32)
    bc = np.asarray(bc, np.float32)

    from concourse.bass_utils import run_bass_kernel_spmd

    nc = _get_module()

    st_host = np.ascontiguousarray(
        supports.transpose(0, 2, 1).astype(np.float16))

    Wru = np.concatenate([Wr, Wu], axis=2)          # [J, 66, 128]
    W_state = Wru[:, IN:, :]                        # [J, 64, 128]
    W_in = Wru[:, :IN, :]                           # [J, 2, 128]
    Wc_state = Wc[:, IN:, :]                        # [J, 64, 64]
    Wc_in = Wc[:, :IN, :]                           # [J, 2, 64]

    wst = np.empty((P, 2 * P), np.float16)
    wca = np.empty((OUT, J * OUT), np.float16)
    for jp in range(2):
        wst[:OUT, jp * P:(jp + 1) * P] = W_state[2 * jp]
        wst[OUT:, jp * P:(jp + 1) * P] = W_state[2 * jp + 1]
    for j in range(J):
        wca[:, j * OUT:(j + 1) * OUT] = Wc_state[j]
    bru = np.concatenate([br, bu]).reshape(2 * OUT, 1).astype(np.float32)

    # host-side input-feature part: AG[j] = S[j] @ input  (3% of FLOPs)
    X = np.ascontiguousarray(
        input.transpose(1, 0, 2).reshape(N, B * IN))
    AG = np.stack([supports[j] @ X for j in range(J)])  # [J, N, B*IN]
    AG4 = AG.reshape(J, N, B, IN)
    ruin_full = np.einsum('jkbf,jfo->bok', AG4, W_in)   # [B, 128, N]
    cin_full = np.einsum('jkbf,jfo->bko', AG4, Wc_in) + bc  # [B, N, 64]

    in_maps = []
    for c in range(NCORES):
        sl = slice(c * BC, (c + 1) * BC)
        st_c = state[sl]                              # [BC, N, OUT]
        xst = np.ascontiguousarray(
            st_c.transpose(1, 0, 2).reshape(N, CB).astype(np.float16))
        stt = np.ascontiguousarray(
            st_c.transpose(0, 2, 1).astype(np.float16))
        ruin = np.ascontiguousarray(ruin_full[sl].astype(np.float16))
        cin = np.ascontiguousarray(
            cin_full[sl].transpose(1, 0, 2).reshape(N, CB)
            .astype(np.float16))
        in_maps.append({
            "st": st_host,
            "xst": xst,
            "stt": stt,
            "ruin": ruin,
            "cin": cin,
            "wst": wst,
            "wca": wca,
            "bru": bru,
        })

    import time
    t0 = time.monotonic()
    res = run_bass_kernel_spmd(nc, in_maps, core_ids=list(range(NCORES)))
    _CACHE["last_wall_s"] = time.monotonic() - t0

    out = np.empty((B, N, OUT), np.float32)
    for c in range(NCORES):
        outk = res.results[c]["outk"]                 # [N, 512] fp16
        for i in range(BC):
            out[c * BC + i] = outk[:, i * OUT:(i + 1) * OUT].astype(
                np.float32)
    return out
